# revision 58
# baseline (speedup 1.0000x reference)
"""Trainium2 Bass kernel for nn_BaseAggregator_31439160607279.

Math (reference):
  af (a,c,f,t), imf (v,c,h,w), split c into k=2 heads of 256 ch.
  sims[a,v,k,hw,t] = sum_c af*imf ; + cls[a,v,k] ; relu ; max over hw ;
  masked mean over t (mask m[a,t] in {0,1}, den = f*sum_t m) ; sum over k.

Strategy:
  - Shard the image dim v=32 across 8 cores (4 images/core); audio replicated.
  - Pack mask-active (a, t) pairs into the matmul M dim -> MT tiles of 128.
  - fp8 e4m3 matmuls in DoubleRow mode (K=256 contraction in one pass):
      MM_S: sims at even hw positions (4 img x 98 "pair-first" cols)
      MM_D: sims of (odd - even) differences (host-precomputed imf diffs;
            per-(a,v,k) cls cancels in differences)
  - Pair-max via max(a,b) = a + relu(b-a):
      ScalarE: R = relu(PSUM_D) -> SBUF f16 (one batched op per M-tile)
      PE:      PSUM_M = I@R (start) then += afp@imf_S (stop)  -> pair maxes
      DVE:     reduce_max over 392 (not 784) elems per (tile, head)
  - cls_sims computed on the host (tiny einsum), gathered per packed row,
    added on GPSIMD; relu on GPSIMD; masked t-sum via matmul with
    0/1 audio-indicator columns accumulated in one PSUM bank.
  - Software pipelining: per block mt emit D(mt)+relu, numdot(mt-7),
    IR/S+reduce(mt-2), cls/relu chain(mt-3).  Persistent SBUF slabs (not
    rotating pool tiles) for R/smraw/sm2/sm3 remove WAR edges; 8 PE warm-up
    matmuls bridge the HAM clock-gate window; input DMAs are few fat
    descriptors ordered so the first tiles' operands land first.
  - Host concatenates per-core outputs along v.
"""

import math
from contextlib import ExitStack

import ml_dtypes
import numpy as np

import concourse.bacc as bacc
import concourse.mybir as mybir
import concourse.tile as tile
from concourse.bass_utils import run_bass_kernel_spmd

# Problem dims (hardcoded per spec)
A, V, C, F, T, H, W = 32, 32, 512, 1, 200, 14, 14
K = 2                    # heads
NCH = C // K             # 256 channels per head
KC = 2                   # contraction sub-tiles (DoubleRow pairs KP rows)
KP = NCH // KC           # 128 = partition contraction per matmul
HW = H * W               # 196
NP = HW // 2             # 98 hw pairs per image
NCORES = 8
VL = V // NCORES         # 4 local images per core
NSC = VL * NP            # 392 = free dim per (tile, head) matmul
PADC = 416               # padded sub-block (DoubleRow needs step%16==0)
SDBLK = K * KC * PADC    # 1664 = cols per sd half (sd-major imf layout)

TRACE = False
LAST_RESULTS = None

_kernel_cache = {}

f32 = mybir.dt.float32
f16 = mybir.dt.float16
fp8 = mybir.dt.float8e4
X = mybir.AxisListType.X
DR = mybir.MatmulPerfMode.DoubleRow
np8 = ml_dtypes.float8_e4m3


def _build(MT: int):
    """Build + compile the per-core Bass program for MT packed-row tiles."""
    nc = bacc.Bacc("TRN2", target_bir_lowering=False, debug=False)

    NAUX = KP + MT * K * VL
    afp_d = nc.dram_tensor("afp", (KP, K * KC * MT * 128), fp8, kind="ExternalInput")
    imf_d = nc.dram_tensor("imf", (KP, 2 * SDBLK), fp8, kind="ExternalInput")
    # aux = [ident (128) | clsb (MT*K*VL)] along the free dim
    aux_d = nc.dram_tensor("aux", (KP, NAUX), f16, kind="ExternalInput")
    maskc_d = nc.dram_tensor("maskc", (KP, MT * A), f16, kind="ExternalInput")
    maskf_d = nc.dram_tensor("maskf", (A, T), f32, kind="ExternalInput")
    outb_d = nc.dram_tensor("outb", (A, K * VL + VL), f32, kind="ExternalOutput")

    with tile.TileContext(nc) as tc, ExitStack() as ctx:
        cst = ctx.enter_context(tc.tile_pool(name="cst", bufs=1))
        psS = ctx.enter_context(tc.tile_pool(name="psS", bufs=3, space="PSUM"))
        psD = ctx.enter_context(tc.tile_pool(name="psD", bufs=2, space="PSUM"))
        psN = ctx.enter_context(tc.tile_pool(name="psN", bufs=1, space="PSUM"))


        # --- persistent SBUF tiles ---
        afp_sb = cst.tile([KP, K * KC * MT * 128], fp8, tag="afp", name="afp_sb")
        imf_sb = cst.tile([KP, 2 * SDBLK], fp8, tag="imf", name="imf_sb")
        aux_sb = cst.tile([KP, NAUX], f16, tag="aux", name="aux_sb")
        ident_sb = aux_sb[:, 0:KP]
        clsb_sb = aux_sb[:, KP:NAUX]
        maskc_sb = cst.tile([KP, MT * A], f16, tag="maskc", name="maskc_sb")
        maskf_sb = cst.tile([A, T], f32, tag="maskf", name="maskf_sb")

        afp_r = afp_sb[:].rearrange("p (k c m) -> p k c m", k=K, c=KC)

        def afp_lhsT(mt, k):
            return afp_r[:, k, :, mt * 128:(mt + 1) * 128]

        def imf_rhs(k, sd):
            off = sd * SDBLK + k * KC * PADC
            view = imf_sb[:, off:off + KC * PADC]
            return view.rearrange("p (c x) -> p c x", c=KC)[:, :, 0:NSC]

        # --- PE warm-up first: keep the PE busy during the input DMA so the
        # HAM clock-gate reaches 8/8 before the real matmuls arrive ---
        warm = cst.tile([KP, 512], f16, tag="warm", name="warm_sb")
        nc.gpsimd.memset(warm[:], 0.0)
        for w in range(6):
            pw = psD.tile([128, 1024], f32, tag="psD", name="ps_warm")
            nc.tensor.matmul(pw[:, 0:512], lhsT=warm[:, 0:128], rhs=warm[:],
                             start=True, stop=True)

        # DMA order: compute-gating transfers first, few fat descriptors.
        # The diff columns (sd=1) gate the first D matmuls -> send them first;
        # sd-major layout keeps each half contiguous per partition.
        nc.sync.dma_start(out=imf_sb[:, SDBLK:2 * SDBLK],
                          in_=imf_d.ap()[:, SDBLK:2 * SDBLK])
        afp_cuts = sorted(set(
            [0, min(2, MT), min(6, MT), min(11, MT), min(17, MT), MT]))
        afp_rd = afp_sb[:].rearrange("p (q m) -> p q m", q=K * KC)
        afp_sd = afp_d.ap().rearrange("p (q m) -> p q m", q=K * KC)

        def afp_chunk_dma(lo, hi):
            nc.sync.dma_start(
                out=afp_rd[:, :, lo * 128:hi * 128],
                in_=afp_sd[:, :, lo * 128:hi * 128],
            )

        chunks = list(zip(afp_cuts[:-1], afp_cuts[1:]))
        afp_chunk_dma(*chunks[0])
        nc.sync.dma_start(out=imf_sb[:, 0:SDBLK], in_=imf_d.ap()[:, 0:SDBLK])
        nc.sync.dma_start(out=aux_sb[:], in_=aux_d.ap())
        nc.sync.dma_start(out=maskf_sb[:], in_=maskf_d.ap())
        if len(chunks) > 1:
            afp_chunk_dma(*chunks[1])
        if len(chunks) > 2:
            afp_chunk_dma(*chunks[2])
        nc.sync.dma_start(out=maskc_sb[:], in_=maskc_d.ap())
        for lo, hi in chunks[3:]:
            afp_chunk_dma(lo, hi)

        # --- main loop ---
        num_ps = psN.tile([A, K * VL], f32, tag="psN", name="ps_numacc")
        den = cst.tile([A, 1], f32, tag="den", name="den")
        rden = cst.tile([A, 1], f32, tag="rden", name="rden")
        # persistent slabs, one slice per M-tile (fewer tiles -> fewer sems)
        R_all = cst.tile([KP, MT * K * NSC], f16, tag="Rall", name="R_all")
        smraw_all = cst.tile([KP, MT * K * VL], f16, tag="smraw", name="smraw_all")
        sm2_all = cst.tile([KP, MT * K * VL], f16, tag="sm2", name="sm2_all")
        sm3_all = cst.tile([KP, MT * K * VL], f16, tag="sm3", name="sm3_all")

        def R_t(mt):
            return R_all[:, mt * K * NSC:(mt + 1) * K * NSC]

        def sl8(t, mt):
            return t[:, mt * K * VL:(mt + 1) * K * VL]

        def emit_D(mt):
            pd = psD.tile([128, 1024], f32, tag="psD", name="ps_D")
            for k in range(K):
                nc.tensor.matmul(
                    pd[:, k * 512:k * 512 + NSC],
                    lhsT=afp_lhsT(mt, k), rhs=imf_rhs(k, 1),
                    perf_mode=DR, start=True, stop=True,
                    skip_group_check=True,
                )
            # relu of both heads' diffs in one ScalarE op
            nc.scalar.activation(
                R_t(mt).rearrange("p (b c) -> p b c", b=K),
                pd[:].rearrange("p (b c) -> p b c", b=K)[:, :, 0:NSC],
                mybir.ActivationFunctionType.Relu,
            )

        def emit_IRS(mt):
            # identity-add of relu'd diffs, then accumulate the even-hw sims
            banks = {}
            for k in range(K):
                ps = psS.tile([128, 512], f32, tag="psS", name="ps_S")
                banks[k] = ps
                nc.tensor.matmul(
                    ps[:, 0:NSC], lhsT=ident_sb,
                    rhs=R_t(mt)[:, k * NSC:(k + 1) * NSC],
                    start=True, stop=False, skip_group_check=True,
                )
            for k in range(K):
                nc.tensor.matmul(
                    banks[k][:, 0:NSC], lhsT=afp_lhsT(mt, k), rhs=imf_rhs(k, 0),
                    perf_mode=DR, start=False, stop=True,
                    skip_group_check=True,
                )
                nc.vector.reduce_max(
                    sl8(smraw_all, mt)[:, k * VL:(k + 1) * VL],
                    banks[k][:, 0:NSC].rearrange("p (i x) -> p i x", i=VL),
                    axis=X,
                )

        def emit_sm(mt, eng=None):
            eng = eng or nc.gpsimd
            eng.tensor_add(sl8(sm2_all, mt), sl8(smraw_all, mt),
                           clsb_sb[:, mt * K * VL:(mt + 1) * K * VL])
            eng.tensor_scalar_max(sl8(sm3_all, mt), sl8(sm2_all, mt), 0.0)

        def emit_numdot(j):
            nc.tensor.matmul(num_ps[:], lhsT=maskc_sb[:, j * A:(j + 1) * A],
                             rhs=sl8(sm3_all, j),
                             start=(j == 0), stop=(j == MT - 1),
                             skip_group_check=True)

        for mt in range(MT):
            emit_D(mt)
            if mt >= 7:
                emit_numdot(mt - 7)
            if mt >= 2:
                emit_IRS(mt - 2)
            if mt >= 3:
                emit_sm(mt - 3)
            if mt == 1:
                nc.vector.reduce_sum(den[:], maskf_sb[:], axis=X)
                nc.vector.reciprocal(rden[:], den[:])

        # tiles whose sm3 is already done can flush before the last IRS work
        for j in range(max(MT - 7, 0), max(MT - 3, 0)):
            emit_numdot(j)
        for j in range(max(MT - 2, 0), MT):
            emit_IRS(j)
        for j in range(max(MT - 3, 0), MT):
            emit_sm(j, eng=nc.vector)
        for j in range(max(MT - 3, 0), MT):
            emit_numdot(j)

        # --- divide, head-sum, out (outb = [outk | outsum]) ---
        outb_sb = cst.tile([A, K * VL + VL], f32, tag="outb", name="outb_sb")
        nc.vector.tensor_scalar_mul(outb_sb[:, 0:K * VL], num_ps[:], rden[:])
        nc.vector.tensor_add(outb_sb[:, K * VL:], outb_sb[:, 0:VL],
                             outb_sb[:, VL:2 * VL])
        nc.sync.dma_start(out=outb_d.ap(), in_=outb_sb[:])

    nc.compile()
    return nc


def prepare_inputs(audio_feats, image_feats, audio_cls, image_cls, audio_mask):
    """Host-side shard + layout prep. Returns (MT, in_maps)."""
    af = np.ascontiguousarray(audio_feats, dtype=np.float32).reshape(
        A, K, KC, KP, T)
    imf = np.ascontiguousarray(image_feats, dtype=np.float32).reshape(
        V, K, KC, KP, HW)
    acls = np.ascontiguousarray(audio_cls, dtype=np.float32).reshape(A, K, NCH)
    icls = np.ascontiguousarray(image_cls, dtype=np.float32).reshape(V, K, NCH)
    mask = np.asarray(audio_mask)
    maskf = np.ascontiguousarray(mask.astype(np.float32))
    cls32 = np.einsum("akc,vkc->avk", acls, icls).astype(np.float32)

    rows_a, rows_t = np.nonzero(mask != 0)
    L = len(rows_a)
    MT = max(1, math.ceil(L / 128))
    LP = MT * 128

    # audio rows, shared by all cores: (K, KC, KP, MT*128) fp8
    af_rows = np.zeros((LP, K, KC, KP), np.float32)
    af_rows[:L] = af[rows_a, :, :, :, rows_t]
    afp = np.ascontiguousarray(
        af_rows.transpose(3, 1, 2, 0).reshape(KP, K * KC * LP)).astype(np8)

    # 0/1 audio-indicator columns for the masked t-sum
    mc = np.zeros((LP, A), np.float16)
    mc[np.arange(L), rows_a] = 1.0
    maskc = np.ascontiguousarray(
        mc.reshape(MT, 128, A).transpose(1, 0, 2).reshape(128, MT * A))

    ident = np.eye(KP, dtype=np.float16)

    # per-core image tensors
    imf_even = imf[..., 0::2]                    # (V,K,KC,KP,98)
    imf_diff = imf[..., 1::2] - imf_even
    in_maps = []
    for ci in range(NCORES):
        vsl = slice(ci * VL, (ci + 1) * VL)
        # sd-major layout [p, sd, k, kc, PADC]: sd=0 -> even sims, 1 -> diffs
        se = imf_even[vsl].transpose(1, 2, 3, 0, 4).reshape(K, KC, KP, NSC)
        sd = imf_diff[vsl].transpose(1, 2, 3, 0, 4).reshape(K, KC, KP, NSC)
        imf_h = np.zeros((2, K, KC, KP, PADC), np.float32)
        imf_h[0, :, :, :, 0:NSC] = se
        imf_h[1, :, :, :, 0:NSC] = sd
        imf_h = np.ascontiguousarray(
            imf_h.transpose(3, 0, 1, 2, 4).reshape(KP, 2 * SDBLK)
        ).astype(np8)

        clsb = np.zeros((LP, K, VL), np.float32)
        clsb[:L] = cls32[rows_a][:, vsl, :].transpose(0, 2, 1)
        clsb_h = (clsb.reshape(MT, 128, K * VL).transpose(1, 0, 2)
                  .reshape(128, MT * K * VL)).astype(np.float16)
        aux = np.ascontiguousarray(np.concatenate([ident, clsb_h], axis=1))

        in_maps.append({
            "afp": afp,
            "imf": imf_h,
            "aux": aux,
            "maskc": maskc,
            "maskf": maskf,
        })
    return MT, in_maps


def get_program(MT: int):
    if MT not in _kernel_cache:
        _kernel_cache[MT] = _build(MT)
    return _kernel_cache[MT]


def kernel(audio_feats, image_feats, audio_cls, image_cls, audio_mask, agg_heads):
    global LAST_RESULTS
    MT, in_maps = prepare_inputs(
        audio_feats, image_feats, audio_cls, image_cls, audio_mask
    )
    nc = get_program(MT)
    res = run_bass_kernel_spmd(nc, in_maps, list(range(NCORES)), trace=TRACE)
    LAST_RESULTS = res
    agg = bool(np.asarray(agg_heads))
    outs = []
    for ci in range(NCORES):
        outb = res.results[ci]["outb"]
        if agg:
            outs.append(outb[:, K * VL:])            # (A, VL)
        else:
            outk = outb[:, 0:K * VL].reshape(A, K, VL)
            outs.append(outk.transpose(0, 2, 1))     # (A, VL, K)
    return np.concatenate(outs, axis=1).astype(np.float32)


# revision 59
# speedup vs baseline: 1.1904x; 1.1904x over previous
"""Trainium2 Bass kernel for nn_BaseAggregator_31439160607279.

Math (reference):
  af (a,c,f,t), imf (v,c,h,w), split c into k=2 heads of 256 ch.
  sims[a,v,k,hw,t] = sum_c af*imf ; + cls[a,v,k] ; relu ; max over hw ;
  masked mean over t (mask m[a,t] in {0,1}, den = f*sum_t m) ; sum over k.

Strategy:
  - Shard the image dim v=32 across 8 cores (4 images/core); audio replicated.
  - Pack mask-active (a, t) pairs into the matmul M dim -> MT tiles of 128.
  - fp8 e4m3 matmuls in DoubleRow mode (K=256 contraction in one pass):
      MM_S: sims at even hw positions (4 img x 98 "pair-first" cols)
      MM_D: sims of (odd - even) differences (host-precomputed imf diffs;
            per-(a,v,k) cls cancels in differences)
  - Pair-max via max(a,b) = a + relu(b-a):
      ScalarE: R = relu(PSUM_D) -> SBUF f16 (one batched op per M-tile)
      PE:      PSUM_M = I@R (start) then += afp@imf_S (stop)  -> pair maxes
      DVE:     reduce_max over 392 (not 784) elems per (tile, head)
  - cls_sims computed on the host (tiny einsum), gathered per packed row,
    added on GPSIMD; relu on GPSIMD; masked t-sum via matmul with
    0/1 audio-indicator columns accumulated in one PSUM bank.
  - Software pipelining: per block mt emit D(mt)+relu, numdot(mt-7),
    IR/S+reduce(mt-2), cls/relu chain(mt-3).  Persistent SBUF slabs (not
    rotating pool tiles) for R/smraw/sm2/sm3 remove WAR edges; 8 PE warm-up
    matmuls bridge the HAM clock-gate window; input DMAs are few fat
    descriptors ordered so the first tiles' operands land first.
  - Host concatenates per-core outputs along v.
"""

import math
from contextlib import ExitStack

import ml_dtypes
import numpy as np

import concourse.bacc as bacc
import concourse.mybir as mybir
import concourse.tile as tile
from concourse.bass_utils import run_bass_kernel_spmd

# Problem dims (hardcoded per spec)
A, V, C, F, T, H, W = 32, 32, 512, 1, 200, 14, 14
K = 2                    # heads
NCH = C // K             # 256 channels per head
KC = 2                   # contraction sub-tiles (DoubleRow pairs KP rows)
KP = NCH // KC           # 128 = partition contraction per matmul
HW = H * W               # 196
NP = HW // 2             # 98 hw pairs per image
NCORES = 8
VL = V // NCORES         # 4 local images per core
NSC = VL * NP            # 392 = free dim per (tile, head) matmul
PADC = 416               # padded sub-block (DoubleRow needs step%16==0)
SDBLK = K * KC * PADC    # 1664 = cols per sd half (sd-major imf layout)

TRACE = False
LAST_RESULTS = None

_kernel_cache = {}

f32 = mybir.dt.float32
f16 = mybir.dt.float16
fp8 = mybir.dt.float8e4
X = mybir.AxisListType.X
DR = mybir.MatmulPerfMode.DoubleRow
np8 = ml_dtypes.float8_e4m3


def _build(MT: int):
    """Build + compile the per-core Bass program for MT packed-row tiles."""
    nc = bacc.Bacc("TRN2", target_bir_lowering=False, debug=False)

    NAUX = KP + MT * K * VL
    afp_d = nc.dram_tensor("afp", (KP, K * KC * MT * 128), fp8, kind="ExternalInput")
    imf_d = nc.dram_tensor("imf", (KP, 2 * SDBLK), fp8, kind="ExternalInput")
    # aux = [ident (128) | clsb (MT*K*VL)] along the free dim
    aux_d = nc.dram_tensor("aux", (KP, NAUX), f16, kind="ExternalInput")
    maskc_d = nc.dram_tensor("maskc", (KP, MT * A), f16, kind="ExternalInput")
    maskf_d = nc.dram_tensor("maskf", (A, T), f32, kind="ExternalInput")
    outb_d = nc.dram_tensor("outb", (A, K * VL + VL), f32, kind="ExternalOutput")

    with tile.TileContext(nc) as tc, ExitStack() as ctx:
        cst = ctx.enter_context(tc.tile_pool(name="cst", bufs=1))
        psS = ctx.enter_context(tc.tile_pool(name="psS", bufs=3, space="PSUM"))
        psD = ctx.enter_context(tc.tile_pool(name="psD", bufs=2, space="PSUM"))
        psN = ctx.enter_context(tc.tile_pool(name="psN", bufs=1, space="PSUM"))


        # --- persistent SBUF tiles ---
        afp_sb = cst.tile([KP, K * KC * MT * 128], fp8, tag="afp", name="afp_sb")
        imf_sb = cst.tile([KP, 2 * SDBLK], fp8, tag="imf", name="imf_sb")
        aux_sb = cst.tile([KP, NAUX], f16, tag="aux", name="aux_sb")
        ident_sb = aux_sb[:, 0:KP]
        clsb_sb = aux_sb[:, KP:NAUX]
        maskc_sb = cst.tile([KP, MT * A], f16, tag="maskc", name="maskc_sb")
        maskf_sb = cst.tile([A, T], f32, tag="maskf", name="maskf_sb")

        afp_r = afp_sb[:].rearrange("p (k c m) -> p k c m", k=K, c=KC)

        def afp_lhsT(mt, k):
            return afp_r[:, k, :, mt * 128:(mt + 1) * 128]

        def imf_rhs(k, sd):
            off = sd * SDBLK + k * KC * PADC
            view = imf_sb[:, off:off + KC * PADC]
            return view.rearrange("p (c x) -> p c x", c=KC)[:, :, 0:NSC]

        # --- PE warm-up first: keep the PE busy during the input DMA so the
        # HAM clock-gate reaches 8/8 before the real matmuls arrive ---
        warm = cst.tile([KP, 512], f16, tag="warm", name="warm_sb")
        nc.gpsimd.memset(warm[:], 0.0)
        for w in range(8):
            pw = psD.tile([128, 1024], f32, tag="psD", name="ps_warm")
            nc.tensor.matmul(pw[:, 0:512], lhsT=warm[:, 0:128], rhs=warm[:],
                             start=True, stop=True)

        # DMA order: compute-gating transfers first, few fat descriptors.
        # The diff columns (sd=1) gate the first D matmuls -> send them first;
        # sd-major layout keeps each half contiguous per partition.
        nc.sync.dma_start(out=imf_sb[:, SDBLK:2 * SDBLK],
                          in_=imf_d.ap()[:, SDBLK:2 * SDBLK])
        afp_cuts = sorted(set(
            [0, min(2, MT), min(6, MT), min(11, MT), min(17, MT), MT]))
        afp_rd = afp_sb[:].rearrange("p (q m) -> p q m", q=K * KC)
        afp_sd = afp_d.ap().rearrange("p (q m) -> p q m", q=K * KC)

        def afp_chunk_dma(lo, hi):
            nc.sync.dma_start(
                out=afp_rd[:, :, lo * 128:hi * 128],
                in_=afp_sd[:, :, lo * 128:hi * 128],
            )

        chunks = list(zip(afp_cuts[:-1], afp_cuts[1:]))
        afp_chunk_dma(*chunks[0])
        nc.sync.dma_start(out=imf_sb[:, 0:SDBLK], in_=imf_d.ap()[:, 0:SDBLK])
        nc.sync.dma_start(out=aux_sb[:], in_=aux_d.ap())
        nc.sync.dma_start(out=maskf_sb[:], in_=maskf_d.ap())
        if len(chunks) > 1:
            afp_chunk_dma(*chunks[1])
        if len(chunks) > 2:
            afp_chunk_dma(*chunks[2])
        nc.sync.dma_start(out=maskc_sb[:], in_=maskc_d.ap())
        for lo, hi in chunks[3:]:
            afp_chunk_dma(lo, hi)

        # --- main loop ---
        num_ps = psN.tile([A, K * VL], f32, tag="psN", name="ps_numacc")
        den = cst.tile([A, 1], f32, tag="den", name="den")
        rden = cst.tile([A, 1], f32, tag="rden", name="rden")
        # persistent slabs, one slice per M-tile (fewer tiles -> fewer sems)
        R_all = cst.tile([KP, MT * K * NSC], f16, tag="Rall", name="R_all")
        smraw_all = cst.tile([KP, MT * K * VL], f16, tag="smraw", name="smraw_all")
        sm2_all = cst.tile([KP, MT * K * VL], f16, tag="sm2", name="sm2_all")
        sm3_all = cst.tile([KP, MT * K * VL], f16, tag="sm3", name="sm3_all")

        def R_t(mt):
            return R_all[:, mt * K * NSC:(mt + 1) * K * NSC]

        def sl8(t, mt):
            return t[:, mt * K * VL:(mt + 1) * K * VL]

        def emit_D(mt):
            pd = psD.tile([128, 1024], f32, tag="psD", name="ps_D")
            for k in range(K):
                nc.tensor.matmul(
                    pd[:, k * 512:k * 512 + NSC],
                    lhsT=afp_lhsT(mt, k), rhs=imf_rhs(k, 1),
                    perf_mode=DR, start=True, stop=True,
                    skip_group_check=True,
                )
            # relu of both heads' diffs in one ScalarE op
            nc.scalar.activation(
                R_t(mt).rearrange("p (b c) -> p b c", b=K),
                pd[:].rearrange("p (b c) -> p b c", b=K)[:, :, 0:NSC],
                mybir.ActivationFunctionType.Relu,
            )

        def emit_IRS(mt):
            # identity-add of relu'd diffs, then accumulate the even-hw sims
            banks = {}
            for k in range(K):
                ps = psS.tile([128, 512], f32, tag="psS", name="ps_S")
                banks[k] = ps
                nc.tensor.matmul(
                    ps[:, 0:NSC], lhsT=ident_sb,
                    rhs=R_t(mt)[:, k * NSC:(k + 1) * NSC],
                    start=True, stop=False, skip_group_check=True,
                )
            for k in range(K):
                nc.tensor.matmul(
                    banks[k][:, 0:NSC], lhsT=afp_lhsT(mt, k), rhs=imf_rhs(k, 0),
                    perf_mode=DR, start=False, stop=True,
                    skip_group_check=True,
                )
                nc.vector.reduce_max(
                    sl8(smraw_all, mt)[:, k * VL:(k + 1) * VL],
                    banks[k][:, 0:NSC].rearrange("p (i x) -> p i x", i=VL),
                    axis=X,
                )

        def emit_sm(mt, eng=None):
            eng = eng or nc.gpsimd
            eng.tensor_add(sl8(sm2_all, mt), sl8(smraw_all, mt),
                           clsb_sb[:, mt * K * VL:(mt + 1) * K * VL])
            eng.tensor_scalar_max(sl8(sm3_all, mt), sl8(sm2_all, mt), 0.0)

        def emit_numdot(j):
            nc.tensor.matmul(num_ps[:], lhsT=maskc_sb[:, j * A:(j + 1) * A],
                             rhs=sl8(sm3_all, j),
                             start=(j == 0), stop=(j == MT - 1),
                             skip_group_check=True)

        for mt in range(MT):
            emit_D(mt)
            if mt >= 7:
                emit_numdot(mt - 7)
            if mt >= 2:
                emit_IRS(mt - 2)
            if mt >= 3:
                emit_sm(mt - 3)
            if mt == 1:
                nc.vector.reduce_sum(den[:], maskf_sb[:], axis=X)
                nc.vector.reciprocal(rden[:], den[:])

        # tiles whose sm3 is already done can flush before the last IRS work
        for j in range(max(MT - 7, 0), max(MT - 3, 0)):
            emit_numdot(j)
        for j in range(max(MT - 2, 0), MT):
            emit_IRS(j)
        for j in range(max(MT - 3, 0), MT):
            emit_sm(j, eng=nc.vector)
        for j in range(max(MT - 3, 0), MT):
            emit_numdot(j)

        # --- divide, head-sum, out (outb = [outk | outsum]) ---
        outb_sb = cst.tile([A, K * VL + VL], f32, tag="outb", name="outb_sb")
        nc.vector.tensor_scalar_mul(outb_sb[:, 0:K * VL], num_ps[:], rden[:])
        nc.vector.tensor_add(outb_sb[:, K * VL:], outb_sb[:, 0:VL],
                             outb_sb[:, VL:2 * VL])
        nc.sync.dma_start(out=outb_d.ap(), in_=outb_sb[:])

    nc.compile()
    return nc


def prepare_inputs(audio_feats, image_feats, audio_cls, image_cls, audio_mask):
    """Host-side shard + layout prep. Returns (MT, in_maps)."""
    af = np.ascontiguousarray(audio_feats, dtype=np.float32).reshape(
        A, K, KC, KP, T)
    imf = np.ascontiguousarray(image_feats, dtype=np.float32).reshape(
        V, K, KC, KP, HW)
    acls = np.ascontiguousarray(audio_cls, dtype=np.float32).reshape(A, K, NCH)
    icls = np.ascontiguousarray(image_cls, dtype=np.float32).reshape(V, K, NCH)
    mask = np.asarray(audio_mask)
    maskf = np.ascontiguousarray(mask.astype(np.float32))
    cls32 = np.einsum("akc,vkc->avk", acls, icls).astype(np.float32)

    rows_a, rows_t = np.nonzero(mask != 0)
    L = len(rows_a)
    MT = max(1, math.ceil(L / 128))
    LP = MT * 128

    # audio rows, shared by all cores: (K, KC, KP, MT*128) fp8
    af_rows = np.zeros((LP, K, KC, KP), np.float32)
    af_rows[:L] = af[rows_a, :, :, :, rows_t]
    afp = np.ascontiguousarray(
        af_rows.transpose(3, 1, 2, 0).reshape(KP, K * KC * LP)).astype(np8)

    # 0/1 audio-indicator columns for the masked t-sum
    mc = np.zeros((LP, A), np.float16)
    mc[np.arange(L), rows_a] = 1.0
    maskc = np.ascontiguousarray(
        mc.reshape(MT, 128, A).transpose(1, 0, 2).reshape(128, MT * A))

    ident = np.eye(KP, dtype=np.float16)

    # per-core image tensors
    imf_even = imf[..., 0::2]                    # (V,K,KC,KP,98)
    imf_diff = imf[..., 1::2] - imf_even
    in_maps = []
    for ci in range(NCORES):
        vsl = slice(ci * VL, (ci + 1) * VL)
        # sd-major layout [p, sd, k, kc, PADC]: sd=0 -> even sims, 1 -> diffs
        se = imf_even[vsl].transpose(1, 2, 3, 0, 4).reshape(K, KC, KP, NSC)
        sd = imf_diff[vsl].transpose(1, 2, 3, 0, 4).reshape(K, KC, KP, NSC)
        imf_h = np.zeros((2, K, KC, KP, PADC), np.float32)
        imf_h[0, :, :, :, 0:NSC] = se
        imf_h[1, :, :, :, 0:NSC] = sd
        imf_h = np.ascontiguousarray(
            imf_h.transpose(3, 0, 1, 2, 4).reshape(KP, 2 * SDBLK)
        ).astype(np8)

        clsb = np.zeros((LP, K, VL), np.float32)
        clsb[:L] = cls32[rows_a][:, vsl, :].transpose(0, 2, 1)
        clsb_h = (clsb.reshape(MT, 128, K * VL).transpose(1, 0, 2)
                  .reshape(128, MT * K * VL)).astype(np.float16)
        aux = np.ascontiguousarray(np.concatenate([ident, clsb_h], axis=1))

        in_maps.append({
            "afp": afp,
            "imf": imf_h,
            "aux": aux,
            "maskc": maskc,
            "maskf": maskf,
        })
    return MT, in_maps


def get_program(MT: int):
    if MT not in _kernel_cache:
        _kernel_cache[MT] = _build(MT)
    return _kernel_cache[MT]


def kernel(audio_feats, image_feats, audio_cls, image_cls, audio_mask, agg_heads):
    global LAST_RESULTS
    MT, in_maps = prepare_inputs(
        audio_feats, image_feats, audio_cls, image_cls, audio_mask
    )
    nc = get_program(MT)
    res = run_bass_kernel_spmd(nc, in_maps, list(range(NCORES)), trace=TRACE)
    LAST_RESULTS = res
    agg = bool(np.asarray(agg_heads))
    outs = []
    for ci in range(NCORES):
        outb = res.results[ci]["outb"]
        if agg:
            outs.append(outb[:, K * VL:])            # (A, VL)
        else:
            outk = outb[:, 0:K * VL].reshape(A, K, VL)
            outs.append(outk.transpose(0, 2, 1))     # (A, VL, K)
    return np.concatenate(outs, axis=1).astype(np.float32)


# revision 60
# speedup vs baseline: 1.2219x; 1.0264x over previous
"""Trainium2 Bass kernel for nn_BaseAggregator_31439160607279.

Math (reference):
  af (a,c,f,t), imf (v,c,h,w), split c into k=2 heads of 256 ch.
  sims[a,v,k,hw,t] = sum_c af*imf ; + cls[a,v,k] ; relu ; max over hw ;
  masked mean over t (mask m[a,t] in {0,1}, den = f*sum_t m) ; sum over k.

Strategy:
  - Shard the image dim v=32 across 8 cores (4 images/core); audio replicated.
  - Pack mask-active (a, t) pairs into the matmul M dim -> MT tiles of 128.
  - fp8 e4m3 matmuls in DoubleRow mode (K=256 contraction in one pass):
      MM_S: sims at even hw positions (4 img x 98 "pair-first" cols)
      MM_D: sims of (odd - even) differences (host-precomputed imf diffs;
            per-(a,v,k) cls cancels in differences)
  - Pair-max via max(a,b) = a + relu(b-a):
      ScalarE: R = relu(PSUM_D) -> SBUF f16 (one batched op per M-tile)
      PE:      PSUM_M = I@R (start) then += afp@imf_S (stop)  -> pair maxes
      DVE:     reduce_max over 392 (not 784) elems per (tile, head)
  - cls_sims computed on the host (tiny einsum), gathered per packed row,
    added on GPSIMD; relu on GPSIMD; masked t-sum via matmul with
    0/1 audio-indicator columns accumulated in one PSUM bank.
  - Software pipelining: per block mt emit D(mt)+relu, numdot(mt-7),
    IR/S+reduce(mt-2), cls/relu chain(mt-3).  Persistent SBUF slabs (not
    rotating pool tiles) for R/smraw/sm2/sm3 remove WAR edges; 8 PE warm-up
    matmuls bridge the HAM clock-gate window; input DMAs are few fat
    descriptors ordered so the first tiles' operands land first.
  - Host concatenates per-core outputs along v.
"""

import math
from contextlib import ExitStack

import ml_dtypes
import numpy as np

import concourse.bacc as bacc
import concourse.mybir as mybir
import concourse.tile as tile
from concourse.bass_utils import run_bass_kernel_spmd

# Problem dims (hardcoded per spec)
A, V, C, F, T, H, W = 32, 32, 512, 1, 200, 14, 14
K = 2                    # heads
NCH = C // K             # 256 channels per head
KC = 2                   # contraction sub-tiles (DoubleRow pairs KP rows)
KP = NCH // KC           # 128 = partition contraction per matmul
HW = H * W               # 196
NP = HW // 2             # 98 hw pairs per image
NCORES = 8
VL = V // NCORES         # 4 local images per core
NSC = VL * NP            # 392 = free dim per (tile, head) matmul

TRACE = False
LAST_RESULTS = None

_kernel_cache = {}

f32 = mybir.dt.float32
f16 = mybir.dt.float16
fp8 = mybir.dt.float8e4
X = mybir.AxisListType.X
DR = mybir.MatmulPerfMode.DoubleRow
np8 = ml_dtypes.float8_e4m3


def _build(MT: int):
    """Build + compile the per-core Bass program for MT packed-row tiles."""
    nc = bacc.Bacc("TRN2", target_bir_lowering=False, debug=False)

    NAUX = KP + MT * K * VL
    afp_d = nc.dram_tensor("afp", (KP, K * KC * MT * 128), fp8, kind="ExternalInput")
    imf_d = nc.dram_tensor("imf", (KP, K * KC * 2 * NSC), fp8, kind="ExternalInput")
    # aux = [ident (128) | clsb (MT*K*VL)] along the free dim
    aux_d = nc.dram_tensor("aux", (KP, NAUX), f16, kind="ExternalInput")
    maskc_d = nc.dram_tensor("maskc", (KP, MT * A), f16, kind="ExternalInput")
    maskf_d = nc.dram_tensor("maskf", (A, T), f32, kind="ExternalInput")
    outb_d = nc.dram_tensor("outb", (A, K * VL + VL), f32, kind="ExternalOutput")

    with tile.TileContext(nc) as tc, ExitStack() as ctx:
        cst = ctx.enter_context(tc.tile_pool(name="cst", bufs=1))
        psS = ctx.enter_context(tc.tile_pool(name="psS", bufs=3, space="PSUM"))
        psD = ctx.enter_context(tc.tile_pool(name="psD", bufs=2, space="PSUM"))
        psN = ctx.enter_context(tc.tile_pool(name="psN", bufs=1, space="PSUM"))


        # --- persistent SBUF tiles ---
        afp_sb = cst.tile([KP, K * KC * MT * 128], fp8, tag="afp", name="afp_sb")
        imf_sb = cst.tile([KP, K * KC * 2 * NSC], fp8, tag="imf", name="imf_sb")
        aux_sb = cst.tile([KP, NAUX], f16, tag="aux", name="aux_sb")
        ident_sb = aux_sb[:, 0:KP]
        clsb_sb = aux_sb[:, KP:NAUX]
        maskc_sb = cst.tile([KP, MT * A], f16, tag="maskc", name="maskc_sb")
        maskf_sb = cst.tile([A, T], f32, tag="maskf", name="maskf_sb")

        afp_r = afp_sb[:].rearrange("p (k c m) -> p k c m", k=K, c=KC)
        imf_r = imf_sb[:].rearrange("p (k c n) -> p k c n", k=K, c=KC)

        def afp_lhsT(mt, k):
            return afp_r[:, k, :, mt * 128:(mt + 1) * 128]

        def imf_rhs(k, sd):
            return imf_r[:, k, :, sd * NSC:(sd + 1) * NSC]

        # --- PE warm-up first: keep the PE busy during the input DMA so the
        # HAM clock-gate reaches 8/8 before the real matmuls arrive ---
        warm = cst.tile([KP, 512], f16, tag="warm", name="warm_sb")
        nc.gpsimd.memset(warm[:], 0.0)
        for w in range(8):
            pw = psD.tile([128, 1024], f32, tag="psD", name="ps_warm")
            nc.tensor.matmul(pw[:, 0:512], lhsT=warm[:, 0:128], rhs=warm[:],
                             start=True, stop=True)

        # DMA order: compute-gating transfers first, few fat descriptors.
        # The diff columns (sd=1) gate the first D matmuls -> send them first.
        imf_rv = imf_sb[:].rearrange("p (q c) -> p q c", q=K * KC)
        imf_dv = imf_d.ap().rearrange("p (q c) -> p q c", q=K * KC)
        nc.sync.dma_start(out=imf_rv[:, :, NSC:2 * NSC],
                          in_=imf_dv[:, :, NSC:2 * NSC])
        afp_cuts = sorted(set(
            [0, min(2, MT), min(6, MT), min(11, MT), min(17, MT), MT]))
        afp_rd = afp_sb[:].rearrange("p (q m) -> p q m", q=K * KC)
        afp_sd = afp_d.ap().rearrange("p (q m) -> p q m", q=K * KC)

        def afp_chunk_dma(lo, hi):
            nc.sync.dma_start(
                out=afp_rd[:, :, lo * 128:hi * 128],
                in_=afp_sd[:, :, lo * 128:hi * 128],
            )

        chunks = list(zip(afp_cuts[:-1], afp_cuts[1:]))
        afp_chunk_dma(*chunks[0])
        nc.sync.dma_start(out=imf_rv[:, :, 0:NSC], in_=imf_dv[:, :, 0:NSC])
        nc.sync.dma_start(out=aux_sb[:], in_=aux_d.ap())
        nc.sync.dma_start(out=maskf_sb[:], in_=maskf_d.ap())
        if len(chunks) > 1:
            afp_chunk_dma(*chunks[1])
        if len(chunks) > 2:
            afp_chunk_dma(*chunks[2])
        nc.sync.dma_start(out=maskc_sb[:], in_=maskc_d.ap())
        for lo, hi in chunks[3:]:
            afp_chunk_dma(lo, hi)

        # --- main loop ---
        num_ps = psN.tile([A, K * VL], f32, tag="psN", name="ps_numacc")
        den = cst.tile([A, 1], f32, tag="den", name="den")
        rden = cst.tile([A, 1], f32, tag="rden", name="rden")
        # persistent slabs, one slice per M-tile (fewer tiles -> fewer sems)
        R_all = cst.tile([KP, MT * K * NSC], f16, tag="Rall", name="R_all")
        smraw_all = cst.tile([KP, MT * K * VL], f16, tag="smraw", name="smraw_all")
        sm2_all = cst.tile([KP, MT * K * VL], f16, tag="sm2", name="sm2_all")
        sm3_all = cst.tile([KP, MT * K * VL], f16, tag="sm3", name="sm3_all")

        def R_t(mt):
            return R_all[:, mt * K * NSC:(mt + 1) * K * NSC]

        def sl8(t, mt):
            return t[:, mt * K * VL:(mt + 1) * K * VL]

        def emit_D(mt):
            pd = psD.tile([128, 1024], f32, tag="psD", name="ps_D")
            for k in range(K):
                nc.tensor.matmul(
                    pd[:, k * 512:k * 512 + NSC],
                    lhsT=afp_lhsT(mt, k), rhs=imf_rhs(k, 1),
                    perf_mode=DR, start=True, stop=True,
                    skip_group_check=True,
                )
            # relu of both heads' diffs in one ScalarE op
            nc.scalar.activation(
                R_t(mt).rearrange("p (b c) -> p b c", b=K),
                pd[:].rearrange("p (b c) -> p b c", b=K)[:, :, 0:NSC],
                mybir.ActivationFunctionType.Relu,
            )

        def emit_IRS(mt):
            # identity-add of relu'd diffs, then accumulate the even-hw sims
            banks = {}
            for k in range(K):
                ps = psS.tile([128, 512], f32, tag="psS", name="ps_S")
                banks[k] = ps
                nc.tensor.matmul(
                    ps[:, 0:NSC], lhsT=ident_sb,
                    rhs=R_t(mt)[:, k * NSC:(k + 1) * NSC],
                    start=True, stop=False, skip_group_check=True,
                )
            for k in range(K):
                nc.tensor.matmul(
                    banks[k][:, 0:NSC], lhsT=afp_lhsT(mt, k), rhs=imf_rhs(k, 0),
                    perf_mode=DR, start=False, stop=True,
                    skip_group_check=True,
                )
                nc.vector.reduce_max(
                    sl8(smraw_all, mt)[:, k * VL:(k + 1) * VL],
                    banks[k][:, 0:NSC].rearrange("p (i x) -> p i x", i=VL),
                    axis=X,
                )

        def emit_sm(mt, eng=None):
            eng = eng or nc.gpsimd
            eng.tensor_add(sl8(sm2_all, mt), sl8(smraw_all, mt),
                           clsb_sb[:, mt * K * VL:(mt + 1) * K * VL])
            eng.tensor_scalar_max(sl8(sm3_all, mt), sl8(sm2_all, mt), 0.0)

        def emit_numdot(j):
            nc.tensor.matmul(num_ps[:], lhsT=maskc_sb[:, j * A:(j + 1) * A],
                             rhs=sl8(sm3_all, j),
                             start=(j == 0), stop=(j == MT - 1),
                             skip_group_check=True)

        for mt in range(MT):
            emit_D(mt)
            if mt >= 7:
                emit_numdot(mt - 7)
            if mt >= 2:
                emit_IRS(mt - 2)
            if mt >= 3:
                emit_sm(mt - 3)
            if mt == 1:
                nc.vector.reduce_sum(den[:], maskf_sb[:], axis=X)
                nc.vector.reciprocal(rden[:], den[:])

        # tiles whose sm3 is already done can flush before the last IRS work
        for j in range(max(MT - 7, 0), max(MT - 3, 0)):
            emit_numdot(j)
        for j in range(max(MT - 2, 0), MT):
            emit_IRS(j)
        for j in range(max(MT - 3, 0), MT):
            emit_sm(j, eng=nc.vector)
        for j in range(max(MT - 3, 0), MT):
            emit_numdot(j)

        # --- divide, head-sum, out (outb = [outk | outsum]) ---
        outb_sb = cst.tile([A, K * VL + VL], f32, tag="outb", name="outb_sb")
        nc.vector.tensor_scalar_mul(outb_sb[:, 0:K * VL], num_ps[:], rden[:])
        nc.vector.tensor_add(outb_sb[:, K * VL:], outb_sb[:, 0:VL],
                             outb_sb[:, VL:2 * VL])
        nc.sync.dma_start(out=outb_d.ap(), in_=outb_sb[:])

    nc.compile()
    return nc


def prepare_inputs(audio_feats, image_feats, audio_cls, image_cls, audio_mask):
    """Host-side shard + layout prep. Returns (MT, in_maps)."""
    af = np.ascontiguousarray(audio_feats, dtype=np.float32).reshape(
        A, K, KC, KP, T)
    imf = np.ascontiguousarray(image_feats, dtype=np.float32).reshape(
        V, K, KC, KP, HW)
    acls = np.ascontiguousarray(audio_cls, dtype=np.float32).reshape(A, K, NCH)
    icls = np.ascontiguousarray(image_cls, dtype=np.float32).reshape(V, K, NCH)
    mask = np.asarray(audio_mask)
    maskf = np.ascontiguousarray(mask.astype(np.float32))
    cls32 = np.einsum("akc,vkc->avk", acls, icls).astype(np.float32)

    rows_a, rows_t = np.nonzero(mask != 0)
    L = len(rows_a)
    MT = max(1, math.ceil(L / 128))
    LP = MT * 128

    # audio rows, shared by all cores: (K, KC, KP, MT*128) fp8
    af_rows = np.zeros((LP, K, KC, KP), np.float32)
    af_rows[:L] = af[rows_a, :, :, :, rows_t]
    afp = np.ascontiguousarray(
        af_rows.transpose(3, 1, 2, 0).reshape(KP, K * KC * LP)).astype(np8)

    # 0/1 audio-indicator columns for the masked t-sum
    mc = np.zeros((LP, A), np.float16)
    mc[np.arange(L), rows_a] = 1.0
    maskc = np.ascontiguousarray(
        mc.reshape(MT, 128, A).transpose(1, 0, 2).reshape(128, MT * A))

    ident = np.eye(KP, dtype=np.float16)

    # per-core image tensors
    imf_even = imf[..., 0::2]                    # (V,K,KC,KP,98)
    imf_diff = imf[..., 1::2] - imf_even
    in_maps = []
    for ci in range(NCORES):
        vsl = slice(ci * VL, (ci + 1) * VL)
        # cols: [k, kc, sd, img, pair] with sd=0 -> even sims, 1 -> diffs
        se = imf_even[vsl].transpose(1, 2, 3, 0, 4).reshape(K, KC, KP, NSC)
        sd = imf_diff[vsl].transpose(1, 2, 3, 0, 4).reshape(K, KC, KP, NSC)
        imf_h = np.concatenate([se[:, :, :, None], sd[:, :, :, None]], axis=3)
        imf_h = np.ascontiguousarray(
            imf_h.transpose(2, 0, 1, 3, 4).reshape(KP, K * KC * 2 * NSC)
        ).astype(np8)

        clsb = np.zeros((LP, K, VL), np.float32)
        clsb[:L] = cls32[rows_a][:, vsl, :].transpose(0, 2, 1)
        clsb_h = (clsb.reshape(MT, 128, K * VL).transpose(1, 0, 2)
                  .reshape(128, MT * K * VL)).astype(np.float16)
        aux = np.ascontiguousarray(np.concatenate([ident, clsb_h], axis=1))

        in_maps.append({
            "afp": afp,
            "imf": imf_h,
            "aux": aux,
            "maskc": maskc,
            "maskf": maskf,
        })
    return MT, in_maps


def get_program(MT: int):
    if MT not in _kernel_cache:
        _kernel_cache[MT] = _build(MT)
    return _kernel_cache[MT]


def kernel(audio_feats, image_feats, audio_cls, image_cls, audio_mask, agg_heads):
    global LAST_RESULTS
    MT, in_maps = prepare_inputs(
        audio_feats, image_feats, audio_cls, image_cls, audio_mask
    )
    nc = get_program(MT)
    res = run_bass_kernel_spmd(nc, in_maps, list(range(NCORES)), trace=TRACE)
    LAST_RESULTS = res
    agg = bool(np.asarray(agg_heads))
    outs = []
    for ci in range(NCORES):
        outb = res.results[ci]["outb"]
        if agg:
            outs.append(outb[:, K * VL:])            # (A, VL)
        else:
            outk = outb[:, 0:K * VL].reshape(A, K, VL)
            outs.append(outk.transpose(0, 2, 1))     # (A, VL, K)
    return np.concatenate(outs, axis=1).astype(np.float32)


# revision 61
# speedup vs baseline: 1.3525x; 1.1069x over previous
"""Trainium2 Bass kernel for nn_BaseAggregator_31439160607279.

Math (reference):
  af (a,c,f,t), imf (v,c,h,w), split c into k=2 heads of 256 ch.
  sims[a,v,k,hw,t] = sum_c af*imf ; + cls[a,v,k] ; relu ; max over hw ;
  masked mean over t (mask m[a,t] in {0,1}, den = f*sum_t m) ; sum over k.

Strategy:
  - Shard the image dim v=32 across 8 cores (4 images/core); audio replicated.
  - Pack mask-active (a, t) pairs into the matmul M dim -> MT tiles of 128.
  - fp8 e4m3 matmuls in DoubleRow mode (K=256 contraction in one pass):
      MM_S: sims at even hw positions (4 img x 98 "pair-first" cols)
      MM_D: sims of (odd - even) differences (host-precomputed imf diffs;
            per-(a,v,k) cls cancels in differences)
  - Pair-max via max(a,b) = a + relu(b-a):
      ScalarE: R = relu(PSUM_D) -> SBUF f16 (one batched op per M-tile)
      PE:      PSUM_M = I@R (start) then += afp@imf_S (stop)  -> pair maxes
      DVE:     reduce_max over 392 (not 784) elems per (tile, head)
  - cls_sims computed on the host (tiny einsum), gathered per packed row,
    added on GPSIMD; relu on GPSIMD; masked t-sum via matmul with
    0/1 audio-indicator columns accumulated in one PSUM bank.
  - Software pipelining: per block mt emit D(mt)+relu, numdot(mt-7),
    IR/S+reduce(mt-2), cls/relu chain(mt-3).  Persistent SBUF slabs (not
    rotating pool tiles) for R/smraw/sm2/sm3 remove WAR edges; 8 PE warm-up
    matmuls bridge the HAM clock-gate window; input DMAs are few fat
    descriptors ordered so the first tiles' operands land first.
  - Host concatenates per-core outputs along v.
"""

import math
from contextlib import ExitStack

import ml_dtypes
import numpy as np

import concourse.bacc as bacc
import concourse.mybir as mybir
import concourse.tile as tile
from concourse.bass_utils import run_bass_kernel_spmd

# Problem dims (hardcoded per spec)
A, V, C, F, T, H, W = 32, 32, 512, 1, 200, 14, 14
K = 2                    # heads
NCH = C // K             # 256 channels per head
KC = 2                   # contraction sub-tiles (DoubleRow pairs KP rows)
KP = NCH // KC           # 128 = partition contraction per matmul
HW = H * W               # 196
NP = HW // 2             # 98 hw pairs per image
NCORES = 8
VL = V // NCORES         # 4 local images per core
NSC = VL * NP            # 392 = free dim per (tile, head) matmul

TRACE = False
LAST_RESULTS = None

_kernel_cache = {}

f32 = mybir.dt.float32
f16 = mybir.dt.float16
fp8 = mybir.dt.float8e4
X = mybir.AxisListType.X
DR = mybir.MatmulPerfMode.DoubleRow
np8 = ml_dtypes.float8_e4m3


def _build(MT: int):
    """Build + compile the per-core Bass program for MT packed-row tiles."""
    nc = bacc.Bacc("TRN2", target_bir_lowering=False, debug=False)

    NAUX = KP + MT * K * VL
    afp_d = nc.dram_tensor("afp", (KP, K * KC * MT * 128), fp8, kind="ExternalInput")
    imf_d = nc.dram_tensor("imf", (KP, K * KC * 2 * NSC), fp8, kind="ExternalInput")
    # aux = [ident (128) | clsb (MT*K*VL)] along the free dim
    aux_d = nc.dram_tensor("aux", (KP, NAUX), f16, kind="ExternalInput")
    P2 = (MT + 1) // 2
    maskc_d = nc.dram_tensor("maskc", (KP, P2 * 2 * A), fp8, kind="ExternalInput")
    maskf_d = nc.dram_tensor("maskf", (A, T), f32, kind="ExternalInput")
    outb_d = nc.dram_tensor("outb", (A, K * VL + VL), f32, kind="ExternalOutput")

    with tile.TileContext(nc) as tc, ExitStack() as ctx:
        cst = ctx.enter_context(tc.tile_pool(name="cst", bufs=1))
        psS = ctx.enter_context(tc.tile_pool(name="psS", bufs=3, space="PSUM"))
        psD = ctx.enter_context(tc.tile_pool(name="psD", bufs=2, space="PSUM"))
        psN = ctx.enter_context(tc.tile_pool(name="psN", bufs=1, space="PSUM"))


        # --- persistent SBUF tiles ---
        afp_sb = cst.tile([KP, K * KC * MT * 128], fp8, tag="afp", name="afp_sb")
        imf_sb = cst.tile([KP, K * KC * 2 * NSC], fp8, tag="imf", name="imf_sb")
        aux_sb = cst.tile([KP, NAUX], f16, tag="aux", name="aux_sb")
        ident_sb = aux_sb[:, 0:KP]
        clsb_sb = aux_sb[:, KP:NAUX]
        maskc_sb = cst.tile([KP, P2 * 2 * A], fp8, tag="maskc", name="maskc_sb")
        maskf_sb = cst.tile([A, T], f32, tag="maskf", name="maskf_sb")

        afp_r = afp_sb[:].rearrange("p (k c m) -> p k c m", k=K, c=KC)
        imf_r = imf_sb[:].rearrange("p (k c n) -> p k c n", k=K, c=KC)

        def afp_lhsT(mt, k):
            return afp_r[:, k, :, mt * 128:(mt + 1) * 128]

        def imf_rhs(k, sd):
            return imf_r[:, k, :, sd * NSC:(sd + 1) * NSC]

        # --- PE warm-up first: keep the PE busy during the input DMA so the
        # HAM clock-gate reaches 8/8 before the real matmuls arrive ---
        warm = cst.tile([KP, 512], f16, tag="warm", name="warm_sb")
        nc.gpsimd.memset(warm[:], 0.0)
        for w in range(8):
            pw = psD.tile([128, 1024], f32, tag="psD", name="ps_warm")
            nc.tensor.matmul(pw[:, 0:512], lhsT=warm[:, 0:128], rhs=warm[:],
                             start=True, stop=True)

        # DMA order: compute-gating transfers first, few fat descriptors.
        # The diff columns (sd=1) gate the first D matmuls -> send them first.
        imf_rv = imf_sb[:].rearrange("p (q c) -> p q c", q=K * KC)
        imf_dv = imf_d.ap().rearrange("p (q c) -> p q c", q=K * KC)
        nc.sync.dma_start(out=imf_rv[:, :, NSC:2 * NSC],
                          in_=imf_dv[:, :, NSC:2 * NSC])
        afp_cuts = sorted(set(
            [0, min(2, MT), min(6, MT), min(11, MT), min(17, MT), MT]))
        afp_rd = afp_sb[:].rearrange("p (q m) -> p q m", q=K * KC)
        afp_sd = afp_d.ap().rearrange("p (q m) -> p q m", q=K * KC)

        def afp_chunk_dma(lo, hi):
            nc.sync.dma_start(
                out=afp_rd[:, :, lo * 128:hi * 128],
                in_=afp_sd[:, :, lo * 128:hi * 128],
            )

        chunks = list(zip(afp_cuts[:-1], afp_cuts[1:]))
        afp_chunk_dma(*chunks[0])
        nc.sync.dma_start(out=imf_rv[:, :, 0:NSC], in_=imf_dv[:, :, 0:NSC])
        nc.sync.dma_start(out=aux_sb[:], in_=aux_d.ap())
        nc.sync.dma_start(out=maskf_sb[:], in_=maskf_d.ap())
        if len(chunks) > 1:
            afp_chunk_dma(*chunks[1])
        if len(chunks) > 2:
            afp_chunk_dma(*chunks[2])
        nc.sync.dma_start(out=maskc_sb[:], in_=maskc_d.ap())
        for lo, hi in chunks[3:]:
            afp_chunk_dma(lo, hi)

        # --- main loop ---
        num_ps = psN.tile([A, K * VL], f32, tag="psN", name="ps_numacc")
        den = cst.tile([A, 1], f32, tag="den", name="den")
        rden = cst.tile([A, 1], f32, tag="rden", name="rden")
        # persistent slabs, one slice per M-tile (fewer tiles -> fewer sems)
        R_all = cst.tile([KP, MT * K * NSC], f16, tag="Rall", name="R_all")
        smraw_all = cst.tile([KP, MT * K * VL], f16, tag="smraw", name="smraw_all")
        sm2_all = cst.tile([KP, MT * K * VL], f16, tag="sm2", name="sm2_all")
        sm3_all = cst.tile([KP, MT * 2 * K * VL], fp8, tag="sm3", name="sm3_all")
        nc.gpsimd.memset(sm3_all[:], 0.0)

        def R_t(mt):
            return R_all[:, mt * K * NSC:(mt + 1) * K * NSC]

        def sl8(t, mt):
            return t[:, mt * K * VL:(mt + 1) * K * VL]

        def emit_D(mt):
            pd = psD.tile([128, 1024], f32, tag="psD", name="ps_D")
            for k in range(K):
                nc.tensor.matmul(
                    pd[:, k * 512:k * 512 + NSC],
                    lhsT=afp_lhsT(mt, k), rhs=imf_rhs(k, 1),
                    perf_mode=DR, start=True, stop=True,
                    skip_group_check=True,
                )
            # relu of both heads' diffs in one ScalarE op
            nc.scalar.activation(
                R_t(mt).rearrange("p (b c) -> p b c", b=K),
                pd[:].rearrange("p (b c) -> p b c", b=K)[:, :, 0:NSC],
                mybir.ActivationFunctionType.Relu,
            )

        def emit_IRS(mt):
            # identity-add of relu'd diffs, then accumulate the even-hw sims
            banks = {}
            for k in range(K):
                ps = psS.tile([128, 512], f32, tag="psS", name="ps_S")
                banks[k] = ps
                nc.tensor.matmul(
                    ps[:, 0:NSC], lhsT=ident_sb,
                    rhs=R_t(mt)[:, k * NSC:(k + 1) * NSC],
                    start=True, stop=False, skip_group_check=True,
                )
            for k in range(K):
                nc.tensor.matmul(
                    banks[k][:, 0:NSC], lhsT=afp_lhsT(mt, k), rhs=imf_rhs(k, 0),
                    perf_mode=DR, start=False, stop=True,
                    skip_group_check=True,
                )
                nc.vector.reduce_max(
                    sl8(smraw_all, mt)[:, k * VL:(k + 1) * VL],
                    banks[k][:, 0:NSC].rearrange("p (i x) -> p i x", i=VL),
                    axis=X,
                )

        def emit_sm(mt, eng=None):
            eng = eng or nc.gpsimd
            eng.tensor_add(sl8(sm2_all, mt), sl8(smraw_all, mt),
                           clsb_sb[:, mt * K * VL:(mt + 1) * K * VL])
            eng.tensor_scalar_max(
                sm3_all[:, mt * 2 * K * VL:mt * 2 * K * VL + K * VL],
                sl8(sm2_all, mt), 0.0)

        maskc_r = maskc_sb[:].rearrange("p (q o a) -> p q o a", q=P2, o=2)
        sm3_r = sm3_all[:].rearrange("p (j x) -> p j x", j=2 * P2)

        def emit_numdot(pr):
            nc.tensor.matmul(num_ps[:], lhsT=maskc_r[:, pr],
                             rhs=sm3_r[:, 2 * pr:2 * pr + 2, 0:K * VL],
                             perf_mode=DR,
                             start=(pr == 0), stop=(pr == P2 - 1),
                             skip_group_check=True)

        for mt in range(MT):
            emit_D(mt)
            if mt >= 8 and mt % 2 == 0:
                emit_numdot((mt - 8) // 2)
            if mt >= 2:
                emit_IRS(mt - 2)
            if mt >= 3:
                emit_sm(mt - 3)
            if mt == 1:
                nc.vector.reduce_sum(den[:], maskf_sb[:], axis=X)
                nc.vector.reciprocal(rden[:], den[:])

        done = (max(0, MT - 1 - 8) // 2 + 1) if MT >= 9 else 0
        for j in range(max(MT - 2, 0), MT):
            emit_IRS(j)
        for j in range(max(MT - 3, 0), MT):
            emit_sm(j, eng=nc.vector)
        for pr in range(done, P2):
            emit_numdot(pr)

        # --- divide, head-sum, out (outb = [outk | outsum]) ---
        outb_sb = cst.tile([A, K * VL + VL], f32, tag="outb", name="outb_sb")
        nc.vector.tensor_scalar_mul(outb_sb[:, 0:K * VL], num_ps[:], rden[:])
        nc.vector.tensor_add(outb_sb[:, K * VL:], outb_sb[:, 0:VL],
                             outb_sb[:, VL:2 * VL])
        nc.sync.dma_start(out=outb_d.ap(), in_=outb_sb[:])

    nc.compile()
    return nc


def prepare_inputs(audio_feats, image_feats, audio_cls, image_cls, audio_mask):
    """Host-side shard + layout prep. Returns (MT, in_maps)."""
    af = np.ascontiguousarray(audio_feats, dtype=np.float32).reshape(
        A, K, KC, KP, T)
    imf = np.ascontiguousarray(image_feats, dtype=np.float32).reshape(
        V, K, KC, KP, HW)
    acls = np.ascontiguousarray(audio_cls, dtype=np.float32).reshape(A, K, NCH)
    icls = np.ascontiguousarray(image_cls, dtype=np.float32).reshape(V, K, NCH)
    mask = np.asarray(audio_mask)
    maskf = np.ascontiguousarray(mask.astype(np.float32))
    cls32 = np.einsum("akc,vkc->avk", acls, icls).astype(np.float32)

    rows_a, rows_t = np.nonzero(mask != 0)
    L = len(rows_a)
    MT = max(1, math.ceil(L / 128))
    LP = MT * 128

    # audio rows, shared by all cores: (K, KC, KP, MT*128) fp8
    af_rows = np.zeros((LP, K, KC, KP), np.float32)
    af_rows[:L] = af[rows_a, :, :, :, rows_t]
    afp = np.ascontiguousarray(
        af_rows.transpose(3, 1, 2, 0).reshape(KP, K * KC * LP)).astype(np8)

    # 0/1 audio-indicator columns for the masked t-sum (DoubleRow pairs)
    P2 = (MT + 1) // 2
    mc = np.zeros((P2 * 2 * 128, A), np.float32)
    mc[np.arange(L), rows_a[:L]] = 1.0
    maskc = np.ascontiguousarray(
        mc.reshape(P2, 2, 128, A).transpose(2, 0, 1, 3)
        .reshape(128, P2 * 2 * A)).astype(np8)

    ident = np.eye(KP, dtype=np.float16)

    # per-core image tensors
    imf_even = imf[..., 0::2]                    # (V,K,KC,KP,98)
    imf_diff = imf[..., 1::2] - imf_even
    in_maps = []
    for ci in range(NCORES):
        vsl = slice(ci * VL, (ci + 1) * VL)
        # cols: [k, kc, sd, img, pair] with sd=0 -> even sims, 1 -> diffs
        se = imf_even[vsl].transpose(1, 2, 3, 0, 4).reshape(K, KC, KP, NSC)
        sd = imf_diff[vsl].transpose(1, 2, 3, 0, 4).reshape(K, KC, KP, NSC)
        imf_h = np.concatenate([se[:, :, :, None], sd[:, :, :, None]], axis=3)
        imf_h = np.ascontiguousarray(
            imf_h.transpose(2, 0, 1, 3, 4).reshape(KP, K * KC * 2 * NSC)
        ).astype(np8)

        clsb = np.zeros((LP, K, VL), np.float32)
        clsb[:L] = cls32[rows_a][:, vsl, :].transpose(0, 2, 1)
        clsb_h = (clsb.reshape(MT, 128, K * VL).transpose(1, 0, 2)
                  .reshape(128, MT * K * VL)).astype(np.float16)
        aux = np.ascontiguousarray(np.concatenate([ident, clsb_h], axis=1))

        in_maps.append({
            "afp": afp,
            "imf": imf_h,
            "aux": aux,
            "maskc": maskc,
            "maskf": maskf,
        })
    return MT, in_maps


def get_program(MT: int):
    if MT not in _kernel_cache:
        _kernel_cache[MT] = _build(MT)
    return _kernel_cache[MT]


def kernel(audio_feats, image_feats, audio_cls, image_cls, audio_mask, agg_heads):
    global LAST_RESULTS
    MT, in_maps = prepare_inputs(
        audio_feats, image_feats, audio_cls, image_cls, audio_mask
    )
    nc = get_program(MT)
    res = run_bass_kernel_spmd(nc, in_maps, list(range(NCORES)), trace=TRACE)
    LAST_RESULTS = res
    agg = bool(np.asarray(agg_heads))
    outs = []
    for ci in range(NCORES):
        outb = res.results[ci]["outb"]
        if agg:
            outs.append(outb[:, K * VL:])            # (A, VL)
        else:
            outk = outb[:, 0:K * VL].reshape(A, K, VL)
            outs.append(outk.transpose(0, 2, 1))     # (A, VL, K)
    return np.concatenate(outs, axis=1).astype(np.float32)


# revision 62
# speedup vs baseline: 1.3547x; 1.0016x over previous
"""Trainium2 Bass kernel for nn_BaseAggregator_31439160607279.

Math (reference):
  af (a,c,f,t), imf (v,c,h,w), split c into k=2 heads of 256 ch.
  sims[a,v,k,hw,t] = sum_c af*imf ; + cls[a,v,k] ; relu ; max over hw ;
  masked mean over t (mask m[a,t] in {0,1}, den = f*sum_t m) ; sum over k.

Strategy:
  - Shard the image dim v=32 across 8 cores (4 images/core); audio replicated.
  - Pack mask-active (a, t) pairs into the matmul M dim -> MT tiles of 128.
  - fp8 e4m3 matmuls in DoubleRow mode (K=256 contraction in one pass):
      MM_S: sims at even hw positions (4 img x 98 "pair-first" cols)
      MM_D: sims of (odd - even) differences (host-precomputed imf diffs;
            per-(a,v,k) cls cancels in differences)
  - Pair-max via max(a,b) = a + relu(b-a):
      ScalarE: R = relu(PSUM_D) -> SBUF f16 (one batched op per M-tile)
      PE:      PSUM_M = I@R (start) then += afp@imf_S (stop)  -> pair maxes
      DVE:     reduce_max over 392 (not 784) elems per (tile, head)
  - cls_sims computed on the host (tiny einsum), gathered per packed row,
    added on GPSIMD; relu on GPSIMD; masked t-sum via matmul with
    0/1 audio-indicator columns accumulated in one PSUM bank.
  - Software pipelining: per block mt emit D(mt)+relu, numdot(mt-7),
    IR/S+reduce(mt-2), cls/relu chain(mt-3).  Persistent SBUF slabs (not
    rotating pool tiles) for R/smraw/sm2/sm3 remove WAR edges; 8 PE warm-up
    matmuls bridge the HAM clock-gate window; input DMAs are few fat
    descriptors ordered so the first tiles' operands land first.
  - Host concatenates per-core outputs along v.
"""

import math
from contextlib import ExitStack

import ml_dtypes
import numpy as np

import concourse.bacc as bacc
import concourse.mybir as mybir
import concourse.tile as tile
from concourse.bass_utils import run_bass_kernel_spmd

# Problem dims (hardcoded per spec)
A, V, C, F, T, H, W = 32, 32, 512, 1, 200, 14, 14
K = 2                    # heads
NCH = C // K             # 256 channels per head
KC = 2                   # contraction sub-tiles (DoubleRow pairs KP rows)
KP = NCH // KC           # 128 = partition contraction per matmul
HW = H * W               # 196
NP = HW // 2             # 98 hw pairs per image
NCORES = 8
VL = V // NCORES         # 4 local images per core
NSC = VL * NP            # 392 = free dim per (tile, head) matmul

TRACE = False
LAST_RESULTS = None

_kernel_cache = {}

f32 = mybir.dt.float32
f16 = mybir.dt.float16
fp8 = mybir.dt.float8e4
X = mybir.AxisListType.X
DR = mybir.MatmulPerfMode.DoubleRow
np8 = ml_dtypes.float8_e4m3


def _build(MT: int):
    """Build + compile the per-core Bass program for MT packed-row tiles."""
    nc = bacc.Bacc("TRN2", target_bir_lowering=False, debug=False)

    NAUX = KP + MT * K * VL
    afp_d = nc.dram_tensor("afp", (KP, K * KC * MT * 128), fp8, kind="ExternalInput")
    imf_d = nc.dram_tensor("imf", (KP, K * KC * 2 * NSC), fp8, kind="ExternalInput")
    # aux = [ident (128) | clsb (MT*K*VL)] along the free dim
    aux_d = nc.dram_tensor("aux", (KP, NAUX), f16, kind="ExternalInput")
    P2 = (MT + 1) // 2
    maskc_d = nc.dram_tensor("maskc", (KP, P2 * 2 * A), fp8, kind="ExternalInput")
    maskf_d = nc.dram_tensor("maskf", (A, T), f32, kind="ExternalInput")
    outb_d = nc.dram_tensor("outb", (A, K * VL + VL), f32, kind="ExternalOutput")

    with tile.TileContext(nc) as tc, ExitStack() as ctx:
        cst = ctx.enter_context(tc.tile_pool(name="cst", bufs=1))
        psS = ctx.enter_context(tc.tile_pool(name="psS", bufs=3, space="PSUM"))
        psD = ctx.enter_context(tc.tile_pool(name="psD", bufs=2, space="PSUM"))
        psN = ctx.enter_context(tc.tile_pool(name="psN", bufs=1, space="PSUM"))


        # --- persistent SBUF tiles ---
        afp_sb = cst.tile([KP, K * KC * MT * 128], fp8, tag="afp", name="afp_sb")
        imf_sb = cst.tile([KP, K * KC * 2 * NSC], fp8, tag="imf", name="imf_sb")
        aux_sb = cst.tile([KP, NAUX], f16, tag="aux", name="aux_sb")
        ident_sb = aux_sb[:, 0:KP]
        clsb_sb = aux_sb[:, KP:NAUX]
        maskc_sb = cst.tile([KP, P2 * 2 * A], fp8, tag="maskc", name="maskc_sb")
        maskf_sb = cst.tile([A, T], f32, tag="maskf", name="maskf_sb")

        afp_r = afp_sb[:].rearrange("p (k c m) -> p k c m", k=K, c=KC)
        imf_r = imf_sb[:].rearrange("p (k c n) -> p k c n", k=K, c=KC)

        def afp_lhsT(mt, k):
            return afp_r[:, k, :, mt * 128:(mt + 1) * 128]

        def imf_rhs(k, sd):
            return imf_r[:, k, :, sd * NSC:(sd + 1) * NSC]

        # --- PE warm-up first: keep the PE busy during the input DMA so the
        # HAM clock-gate reaches 8/8 before the real matmuls arrive ---
        warm = cst.tile([KP, 512], f16, tag="warm", name="warm_sb")
        nc.gpsimd.memset(warm[:], 0.0)
        for w in range(8):
            pw = psD.tile([128, 1024], f32, tag="psD", name="ps_warm")
            nc.tensor.matmul(pw[:, 0:512], lhsT=warm[:, 0:128], rhs=warm[:],
                             start=True, stop=True)

        # DMA order: compute-gating transfers first, few fat descriptors.
        # The diff columns (sd=1) gate the first D matmuls -> send them first.
        imf_rv = imf_sb[:].rearrange("p (q c) -> p q c", q=K * KC)
        imf_dv = imf_d.ap().rearrange("p (q c) -> p q c", q=K * KC)
        nc.sync.dma_start(out=imf_rv[:, :, NSC:2 * NSC],
                          in_=imf_dv[:, :, NSC:2 * NSC])
        afp_cuts = sorted(set(
            [0, min(2, MT), min(6, MT), min(11, MT), min(17, MT), MT]))
        afp_rd = afp_sb[:].rearrange("p (q m) -> p q m", q=K * KC)
        afp_sd = afp_d.ap().rearrange("p (q m) -> p q m", q=K * KC)

        def afp_chunk_dma(lo, hi):
            nc.sync.dma_start(
                out=afp_rd[:, :, lo * 128:hi * 128],
                in_=afp_sd[:, :, lo * 128:hi * 128],
            )

        chunks = list(zip(afp_cuts[:-1], afp_cuts[1:]))
        afp_chunk_dma(*chunks[0])
        nc.sync.dma_start(out=imf_rv[:, :, 0:NSC], in_=imf_dv[:, :, 0:NSC])
        nc.sync.dma_start(out=aux_sb[:], in_=aux_d.ap())
        nc.sync.dma_start(out=maskf_sb[:], in_=maskf_d.ap())
        if len(chunks) > 1:
            afp_chunk_dma(*chunks[1])
        if len(chunks) > 2:
            afp_chunk_dma(*chunks[2])
        nc.sync.dma_start(out=maskc_sb[:], in_=maskc_d.ap())
        for lo, hi in chunks[3:]:
            afp_chunk_dma(lo, hi)

        # --- main loop ---
        num_ps = psN.tile([A, K * VL], f32, tag="psN", name="ps_numacc")
        den = cst.tile([A, 1], f32, tag="den", name="den")
        rden = cst.tile([A, 1], f32, tag="rden", name="rden")
        # persistent slabs, one slice per M-tile (fewer tiles -> fewer sems)
        R_all = cst.tile([KP, MT * K * NSC], f16, tag="Rall", name="R_all")
        smraw_all = cst.tile([KP, MT * K * VL], f16, tag="smraw", name="smraw_all")
        sm2_all = cst.tile([KP, MT * K * VL], f16, tag="sm2", name="sm2_all")
        sm3_all = cst.tile([KP, MT * 2 * K * VL], fp8, tag="sm3", name="sm3_all")
        nc.gpsimd.memset(sm3_all[:], 0.0)

        def R_t(mt):
            return R_all[:, mt * K * NSC:(mt + 1) * K * NSC]

        def sl8(t, mt):
            return t[:, mt * K * VL:(mt + 1) * K * VL]

        def emit_D(mt):
            pd = psD.tile([128, 1024], f32, tag="psD", name="ps_D")
            for k in range(K):
                nc.tensor.matmul(
                    pd[:, k * 512:k * 512 + NSC],
                    lhsT=afp_lhsT(mt, k), rhs=imf_rhs(k, 1),
                    perf_mode=DR, start=True, stop=True,
                    skip_group_check=True,
                )
            # relu of both heads' diffs in one ScalarE op
            nc.scalar.activation(
                R_t(mt).rearrange("p (b c) -> p b c", b=K),
                pd[:].rearrange("p (b c) -> p b c", b=K)[:, :, 0:NSC],
                mybir.ActivationFunctionType.Relu,
            )

        def emit_IRS(mt):
            # identity-add of relu'd diffs, then accumulate the even-hw sims
            banks = {}
            for k in range(K):
                ps = psS.tile([128, 512], f32, tag="psS", name="ps_S")
                banks[k] = ps
                nc.tensor.matmul(
                    ps[:, 0:NSC], lhsT=ident_sb,
                    rhs=R_t(mt)[:, k * NSC:(k + 1) * NSC],
                    start=True, stop=False, skip_group_check=True,
                )
            for k in range(K):
                nc.tensor.matmul(
                    banks[k][:, 0:NSC], lhsT=afp_lhsT(mt, k), rhs=imf_rhs(k, 0),
                    perf_mode=DR, start=False, stop=True,
                    skip_group_check=True,
                )
                nc.vector.reduce_max(
                    sl8(smraw_all, mt)[:, k * VL:(k + 1) * VL],
                    banks[k][:, 0:NSC].rearrange("p (i x) -> p i x", i=VL),
                    axis=X,
                )

        def emit_sm(mt, eng=None):
            eng = eng or nc.gpsimd
            eng.tensor_add(sl8(sm2_all, mt), sl8(smraw_all, mt),
                           clsb_sb[:, mt * K * VL:(mt + 1) * K * VL])
            eng.tensor_scalar_max(
                sm3_all[:, mt * 2 * K * VL:mt * 2 * K * VL + K * VL],
                sl8(sm2_all, mt), 0.0)

        maskc_r = maskc_sb[:].rearrange("p (q o a) -> p q o a", q=P2, o=2)
        sm3_r = sm3_all[:].rearrange("p (j x) -> p j x", j=2 * P2)

        def emit_numdot(pr):
            nc.tensor.matmul(num_ps[:], lhsT=maskc_r[:, pr],
                             rhs=sm3_r[:, 2 * pr:2 * pr + 2, 0:K * VL],
                             perf_mode=DR,
                             start=(pr == 0), stop=(pr == P2 - 1),
                             skip_group_check=True)

        for mt in range(MT):
            emit_D(mt)
            if mt >= 8 and mt % 2 == 0:
                emit_numdot((mt - 8) // 2)
            if mt >= 2:
                emit_IRS(mt - 2)
            if mt >= 3:
                emit_sm(mt - 3)
            if mt == 1:
                nc.vector.reduce_sum(den[:], maskf_sb[:], axis=X)
                nc.vector.reciprocal(rden[:], den[:])

        done = (max(0, MT - 1 - 8) // 2 + 1) if MT >= 9 else 0
        # pairs whose sm3 inputs are already produced in-loop flush first
        early = min(P2, max(done, (MT - 3) // 2))
        for pr in range(done, early):
            emit_numdot(pr)
        for j in range(max(MT - 2, 0), MT):
            emit_IRS(j)
        for j in range(max(MT - 3, 0), MT):
            emit_sm(j, eng=nc.vector)
        for pr in range(early, P2):
            emit_numdot(pr)

        # --- divide, head-sum, out (outb = [outk | outsum]) ---
        outb_sb = cst.tile([A, K * VL + VL], f32, tag="outb", name="outb_sb")
        nc.vector.tensor_scalar_mul(outb_sb[:, 0:K * VL], num_ps[:], rden[:])
        nc.vector.tensor_add(outb_sb[:, K * VL:], outb_sb[:, 0:VL],
                             outb_sb[:, VL:2 * VL])
        nc.sync.dma_start(out=outb_d.ap(), in_=outb_sb[:])

    nc.compile()
    return nc


def prepare_inputs(audio_feats, image_feats, audio_cls, image_cls, audio_mask):
    """Host-side shard + layout prep. Returns (MT, in_maps)."""
    af = np.ascontiguousarray(audio_feats, dtype=np.float32).reshape(
        A, K, KC, KP, T)
    imf = np.ascontiguousarray(image_feats, dtype=np.float32).reshape(
        V, K, KC, KP, HW)
    acls = np.ascontiguousarray(audio_cls, dtype=np.float32).reshape(A, K, NCH)
    icls = np.ascontiguousarray(image_cls, dtype=np.float32).reshape(V, K, NCH)
    mask = np.asarray(audio_mask)
    maskf = np.ascontiguousarray(mask.astype(np.float32))
    cls32 = np.einsum("akc,vkc->avk", acls, icls).astype(np.float32)

    rows_a, rows_t = np.nonzero(mask != 0)
    L = len(rows_a)
    MT = max(1, math.ceil(L / 128))
    LP = MT * 128

    # audio rows, shared by all cores: (K, KC, KP, MT*128) fp8
    af_rows = np.zeros((LP, K, KC, KP), np.float32)
    af_rows[:L] = af[rows_a, :, :, :, rows_t]
    afp = np.ascontiguousarray(
        af_rows.transpose(3, 1, 2, 0).reshape(KP, K * KC * LP)).astype(np8)

    # 0/1 audio-indicator columns for the masked t-sum (DoubleRow pairs)
    P2 = (MT + 1) // 2
    mc = np.zeros((P2 * 2 * 128, A), np.float32)
    mc[np.arange(L), rows_a[:L]] = 1.0
    maskc = np.ascontiguousarray(
        mc.reshape(P2, 2, 128, A).transpose(2, 0, 1, 3)
        .reshape(128, P2 * 2 * A)).astype(np8)

    ident = np.eye(KP, dtype=np.float16)

    # per-core image tensors
    imf_even = imf[..., 0::2]                    # (V,K,KC,KP,98)
    imf_diff = imf[..., 1::2] - imf_even
    in_maps = []
    for ci in range(NCORES):
        vsl = slice(ci * VL, (ci + 1) * VL)
        # cols: [k, kc, sd, img, pair] with sd=0 -> even sims, 1 -> diffs
        se = imf_even[vsl].transpose(1, 2, 3, 0, 4).reshape(K, KC, KP, NSC)
        sd = imf_diff[vsl].transpose(1, 2, 3, 0, 4).reshape(K, KC, KP, NSC)
        imf_h = np.concatenate([se[:, :, :, None], sd[:, :, :, None]], axis=3)
        imf_h = np.ascontiguousarray(
            imf_h.transpose(2, 0, 1, 3, 4).reshape(KP, K * KC * 2 * NSC)
        ).astype(np8)

        clsb = np.zeros((LP, K, VL), np.float32)
        clsb[:L] = cls32[rows_a][:, vsl, :].transpose(0, 2, 1)
        clsb_h = (clsb.reshape(MT, 128, K * VL).transpose(1, 0, 2)
                  .reshape(128, MT * K * VL)).astype(np.float16)
        aux = np.ascontiguousarray(np.concatenate([ident, clsb_h], axis=1))

        in_maps.append({
            "afp": afp,
            "imf": imf_h,
            "aux": aux,
            "maskc": maskc,
            "maskf": maskf,
        })
    return MT, in_maps


def get_program(MT: int):
    if MT not in _kernel_cache:
        _kernel_cache[MT] = _build(MT)
    return _kernel_cache[MT]


def kernel(audio_feats, image_feats, audio_cls, image_cls, audio_mask, agg_heads):
    global LAST_RESULTS
    MT, in_maps = prepare_inputs(
        audio_feats, image_feats, audio_cls, image_cls, audio_mask
    )
    nc = get_program(MT)
    res = run_bass_kernel_spmd(nc, in_maps, list(range(NCORES)), trace=TRACE)
    LAST_RESULTS = res
    agg = bool(np.asarray(agg_heads))
    outs = []
    for ci in range(NCORES):
        outb = res.results[ci]["outb"]
        if agg:
            outs.append(outb[:, K * VL:])            # (A, VL)
        else:
            outk = outb[:, 0:K * VL].reshape(A, K, VL)
            outs.append(outk.transpose(0, 2, 1))     # (A, VL, K)
    return np.concatenate(outs, axis=1).astype(np.float32)
